# revision 7
# baseline (speedup 1.0000x reference)
"""Trainium2 Bass kernel for nn_ActionScoringModel (LRU + max-pool + tanh MLP).

Strategy: data-parallel over batch (64 = 8 cores x 8 batches). No collectives.
Per core (V2 pipeline):
  - obs cast to bf16 on host; obsT [d, s] produced by DMA-xbar transpose
    straight from DRAM (no PE transposes, no PSUM->SBUF copies)
  - u = statA @ obsT with k-major stationary reuse (PSUM, 4 banks)
  - rotate-in: t1 = u (.) cos, t2 = u (.) sin' on DVE (bf16);
    v = I@t1 + P@t2 on PE (partition swap folded into permutation stationary)
  - scan decimation x4: w_m = rho v_{2m-1} + v_{2m} (stt), w2 likewise;
    hardware scan runs only length-512 chains (alternating DVE/gpsimd),
    then stt fixups reconstruct odd positions. Output g in plane-permuted
    order [s%4 = 0,2,1,3] which is fine because latent = max over s.
  - rotate-out: p1 = g (.) cos2P, p2 = g (.) sin2P (plane-permuted tables)
  - y = CM1@p1 + CM2@p2 + D@obsT(strided plane view) per plane; max-reduce
    on gpsimd; tanh MLP head on [latent, act] with DMA-transposed actT.
"""

import sys
import numpy as np
from contextlib import ExitStack

for _p in ("/opt/trn_rl_repo",):
    if _p not in sys.path:
        sys.path.insert(0, _p)

import ml_dtypes
import concourse.bass as bass
import concourse.tile as tile
from concourse import bacc, mybir
from concourse.bass_utils import run_bass_kernel_spmd

BF16 = mybir.dt.bfloat16
F32 = mybir.dt.float32

B_, S_, A_, D_IN, H_, D_OUT, D_MLP = 64, 2048, 128, 384, 64, 64, 64
NCORES = 8
NB = B_ // NCORES          # 8 batches per core
NSB = S_ // 512            # 4 s-blocks of 512
NDC = D_IN // 128          # 3 d-chunks
SQ = S_ // 4               # 512, plane width / scan length


def _build_nc():
    nc = bacc.Bacc("TRN2", target_bir_lowering=False, debug=False,
                   num_devices=1)

    # ---- DRAM I/O ----
    obs_d = nc.dram_tensor("obs", [NB, S_, D_IN], BF16, kind="ExternalInput").ap()
    act_d = nc.dram_tensor("act", [NB, A_, D_IN], BF16, kind="ExternalInput").ap()
    cosS_d = nc.dram_tensor("cosS", [128, S_], BF16, kind="ExternalInput").ap()
    sinpm2_d = nc.dram_tensor("sinpm2", [128, S_], BF16, kind="ExternalInput").ap()
    cos2P_d = nc.dram_tensor("cos2P", [128, S_], BF16, kind="ExternalInput").ap()
    sin2P_d = nc.dram_tensor("sin2P", [128, S_], BF16, kind="ExternalInput").ap()
    rho4_d = nc.dram_tensor("rho4", [128, SQ], F32, kind="ExternalInput").ap()
    rho1_d = nc.dram_tensor("rho1", [128, 1], F32, kind="ExternalInput").ap()
    rho2_d = nc.dram_tensor("rho2", [128, 1], F32, kind="ExternalInput").ap()
    statA_d = nc.dram_tensor("statA", [NDC, 128, 128], BF16, kind="ExternalInput").ap()
    permP_d = nc.dram_tensor("permP", [128, 128], BF16, kind="ExternalInput").ap()
    ident_d = nc.dram_tensor("ident", [128, 128], BF16, kind="ExternalInput").ap()
    statD_d = nc.dram_tensor("statD", [NDC, 128, D_OUT], BF16, kind="ExternalInput").ap()
    cm1_d = nc.dram_tensor("cm1", [128, D_OUT], BF16, kind="ExternalInput").ap()
    cm2_d = nc.dram_tensor("cm2", [128, D_OUT], BF16, kind="ExternalInput").ap()
    w1lat_d = nc.dram_tensor("w1lat", [64, 64], BF16, kind="ExternalInput").ap()
    w1act_d = nc.dram_tensor("w1act", [NDC, 128, D_MLP], BF16, kind="ExternalInput").ap()
    w2_d = nc.dram_tensor("w2", [64, 32], BF16, kind="ExternalInput").ap()
    w3_d = nc.dram_tensor("w3", [32, 1], BF16, kind="ExternalInput").ap()
    b1_d = nc.dram_tensor("b1", [64, 1], F32, kind="ExternalInput").ap()
    b2_d = nc.dram_tensor("b2", [32, 1], F32, kind="ExternalInput").ap()
    b3_d = nc.dram_tensor("b3", [1, 1], F32, kind="ExternalInput").ap()
    out_d = nc.dram_tensor("out", [1, NB * A_], F32, kind="ExternalOutput").ap()

    MULT = mybir.AluOpType.mult
    ADD = mybir.AluOpType.add
    MAX = mybir.AluOpType.max
    TANH = mybir.ActivationFunctionType.Tanh
    X = mybir.AxisListType.X

    with tile.TileContext(nc) as tc, ExitStack() as ctx:
        const = ctx.enter_context(tc.tile_pool(name="const", bufs=1))
        obsT_pool = ctx.enter_context(tc.tile_pool(name="obsT", bufs=2))
        work = ctx.enter_context(tc.tile_pool(name="work", bufs=2))
        tpool = ctx.enter_context(tc.tile_pool(name="tpool", bufs=3))
        pUA = ctx.enter_context(tc.tile_pool(name="pUA", bufs=1, space="PSUM"))
        pV = ctx.enter_context(tc.tile_pool(name="pV", bufs=2, space="PSUM"))
        pY = ctx.enter_context(tc.tile_pool(name="pY", bufs=2, space="PSUM"))
        small = ctx.enter_context(tc.tile_pool(name="small", bufs=1))

        def load_const(ap_d, shape, dtype, suffix=""):
            nm = f"c_{ap_d.tensor.name}{suffix}"
            t = const.tile(shape, dtype, tag=nm, name=nm)
            nc.scalar.dma_start(out=t[:], in_=ap_d)
            return t

        cosS = load_const(cosS_d, [128, S_], BF16)
        sinpm2 = load_const(sinpm2_d, [128, S_], BF16)
        cos2P = load_const(cos2P_d, [128, S_], BF16)
        sin2P = load_const(sin2P_d, [128, S_], BF16)
        rho4 = load_const(rho4_d, [128, SQ], F32)
        rho1 = load_const(rho1_d, [128, 1], F32)
        rho2 = load_const(rho2_d, [128, 1], F32)
        statA = [load_const(statA_d[k], [128, 128], BF16, f"{k}") for k in range(NDC)]
        permP = load_const(permP_d, [128, 128], BF16)
        ident = load_const(ident_d, [128, 128], BF16)
        statD = [load_const(statD_d[k], [128, D_OUT], BF16, f"{k}") for k in range(NDC)]
        cm1 = load_const(cm1_d, [128, D_OUT], BF16)
        cm2 = load_const(cm2_d, [128, D_OUT], BF16)
        w1lat = load_const(w1lat_d, [64, 64], BF16)
        w1act = [load_const(w1act_d[k], [128, D_MLP], BF16, f"{k}") for k in range(NDC)]
        w2 = load_const(w2_d, [64, 32], BF16)
        w3 = load_const(w3_d, [32, 1], BF16)
        b1 = load_const(b1_d, [64, 1], F32)
        b2 = load_const(b2_d, [32, 1], F32)
        b3 = load_const(b3_d, [1, 1], F32)

        lat = small.tile([64, NB], F32)          # latent columns (max over s)
        latb = small.tile([64, NB], BF16)

        PLANE_OFF = (0, 2, 1, 3)   # s offset (mod 4) of each g/p plane

        # ---------------- main loop over local batches ----------------
        for b in range(NB):
            # obsT[k] [128, S] via DMA xbar transpose straight from DRAM
            obsT = [obsT_pool.tile([128, S_], BF16, tag=f"obsT{k}",
                                   name=f"obsT{k}")
                    for k in range(NDC)]
            for k in range(NDC):
                nc.sync.dma_start_transpose(
                    out=obsT[k][:], in_=obs_d[b, :, k * 128:(k + 1) * 128])

            # u = statA @ obsT, k-major over 2 waves of 2 blocks
            uA = [None] * NSB
            for w0 in range(0, NSB, 2):
                for k in range(NDC):
                    for i in range(w0, w0 + 2):
                        if k == 0:
                            uA[i] = pUA.tile([128, 512], F32, tag=f"uA{i % 4}", name=f"uA{i % 4}")
                        nc.tensor.matmul(
                            out=uA[i][:], lhsT=statA[k][:],
                            rhs=obsT[k][:, i * 512:(i + 1) * 512],
                            start=(k == 0), stop=(k == NDC - 1))

            # rotate-in: uAc (bf16 copy on Act), t1/t2 on DVE,
            # v = I@t1 + P@t2 on PE -> PSUM, Act copies to v_sbuf (padded)
            v = work.tile([128, S_ + 1], BF16, tag="v", name="v")
            nc.gpsimd.memset(v[:, 0:1], 0.0)
            for i in range(NSB):
                sl = slice(i * 512, (i + 1) * 512)
                uAc = tpool.tile([128, 512], BF16, tag="uAc", name="uAc")
                nc.scalar.copy(out=uAc[:], in_=uA[i][:])
                t1 = tpool.tile([128, 512], BF16, tag="t1", name="t1")
                t2 = tpool.tile([128, 512], BF16, tag="t2", name="t2")
                nc.vector.tensor_tensor(out=t1[:], in0=uAc[:], in1=cosS[:, sl],
                                        op=MULT)
                nc.gpsimd.tensor_tensor(out=t2[:], in0=uAc[:], in1=sinpm2[:, sl],
                                        op=MULT)
                vps = pV.tile([128, 512], F32, tag="vps", name="vps")
                if i % 2 == 0:
                    nc.tensor.matmul(out=vps[:], lhsT=ident[:], rhs=t1[:],
                                     start=True, stop=False)
                    nc.tensor.matmul(out=vps[:], lhsT=permP[:], rhs=t2[:],
                                     start=False, stop=True)
                else:
                    nc.tensor.matmul(out=vps[:], lhsT=permP[:], rhs=t2[:],
                                     start=True, stop=False)
                    nc.tensor.matmul(out=vps[:], lhsT=ident[:], rhs=t1[:],
                                     start=False, stop=True)
                nc.scalar.copy(out=v[:, 1 + i * 512:1 + (i + 1) * 512],
                               in_=vps[:])

            # decimation: wE_m = rho*v_{2m-1} + v_{2m}; w2 likewise with rho^2
            wE = work.tile([128, S_ // 2 + 1], BF16, tag="wE", name="wE")
            nc.gpsimd.memset(wE[:, 0:1], 0.0)
            v_lo = v[:, 0:S_].rearrange("p (n f) -> p f n", f=2)[:, 0]
            v_hi = v[:, 1:S_ + 1].rearrange("p (n f) -> p f n", f=2)[:, 0]
            nc.vector.scalar_tensor_tensor(
                out=wE[:, 1:S_ // 2 + 1], in0=v_lo, scalar=rho1[:], in1=v_hi,
                op0=MULT, op1=ADD)
            w2t = work.tile([128, SQ], BF16, tag="w2t", name="w2t")
            wE_lo = wE[:, 0:S_ // 2].rearrange("p (n f) -> p f n", f=2)[:, 0]
            wE_hi = wE[:, 1:S_ // 2 + 1].rearrange("p (n f) -> p f n", f=2)[:, 0]
            nc.vector.scalar_tensor_tensor(
                out=w2t[:], in0=wE_lo, scalar=rho2[:], in1=wE_hi,
                op0=MULT, op1=ADD)

            # length-512 scan (alternate DVE / gpsimd across batches)
            g = work.tile([128, S_], BF16, tag="g", name="g")
            nc.vector.tensor_tensor_scan(out=g[:, 0:SQ], data0=rho4[:],
                                   data1=w2t[:], initial=0.0,
                                   op0=MULT, op1=ADD)

            # fixups: gEO = rho2*gEE + wE_odd ; gOE/gOO from v
            wE_odd = wE[:, 1:S_ // 2 + 1].rearrange("p (n f) -> p f n", f=2)[:, 1]
            nc.vector.scalar_tensor_tensor(
                out=g[:, SQ:2 * SQ], in0=g[:, 0:SQ], scalar=rho2[:],
                in1=wE_odd, op0=MULT, op1=ADD)
            v4 = v[:, 0:S_].rearrange("p (n f) -> p f n", f=4)
            v4b = v[:, 1:S_ + 1].rearrange("p (n f) -> p f n", f=4)
            nc.vector.scalar_tensor_tensor(
                out=g[:, 2 * SQ:3 * SQ], in0=g[:, 0:SQ], scalar=rho1[:],
                in1=v4[:, 2], op0=MULT, op1=ADD)
            nc.vector.scalar_tensor_tensor(
                out=g[:, 3 * SQ:4 * SQ], in0=g[:, SQ:2 * SQ], scalar=rho1[:],
                in1=v4b[:, 3], op0=MULT, op1=ADD)

            # rotate-out (plane-permuted tables)
            p1 = work.tile([128, S_], BF16, tag="p1", name="p1")
            p2 = work.tile([128, S_], BF16, tag="p2", name="p2")
            nc.vector.tensor_tensor(out=p1[:], in0=g[:], in1=cos2P[:], op=MULT)
            nc.vector.tensor_tensor(out=p2[:], in0=g[:], in1=sin2P[:], op=MULT)

            # y = CM1@p1 + CM2@p2 + D@obsT(plane view); latent = max_s y
            ymax = small.tile([64, NSB], F32, tag=f"ymax", name="ymax")
            for pl in range(4):
                sl = slice(pl * SQ, (pl + 1) * SQ)
                off = PLANE_OFF[pl]
                py = pY.tile([64, 512], F32, tag="pY", name="pY")
                nc.tensor.matmul(out=py[:], lhsT=cm1[:], rhs=p1[:, sl],
                                 start=True, stop=False)
                nc.tensor.matmul(out=py[:], lhsT=cm2[:], rhs=p2[:, sl],
                                 start=False, stop=False)
                for k in range(NDC):
                    obsP = obsT[k][:].rearrange("p (n f) -> p f n", f=4)[:, off]
                    nc.tensor.matmul(out=py[:], lhsT=statD[k][:],
                                     rhs=obsP, start=False,
                                     stop=(k == NDC - 1))
                nc.vector.tensor_reduce(out=ymax[:, pl:pl + 1], in_=py[:],
                                        axis=X, op=MAX)
            nc.vector.tensor_reduce(out=lat[:, b:b + 1], in_=ymax[:],
                                    axis=X, op=MAX)

        # ---------------- MLP head ----------------
        nc.vector.tensor_copy(out=latb[:], in_=lat[:])

        platW = pY.tile([64, 512], F32, tag="pY", name="pY")
        nc.tensor.matmul(out=platW[:, :NB], lhsT=w1lat[:], rhs=latb[:],
                         start=True, stop=True)
        latWb = small.tile([64, NB], F32)
        nc.vector.tensor_scalar(out=latWb[:], in0=platW[:, :NB],
                                scalar1=b1[:], scalar2=None, op0=ADD)

        # actT via DMA xbar transpose
        actT = [small.tile([128, NB * A_], BF16, tag=f"actT{k}",
                           name=f"actT{k}") for k in range(NDC)]
        for k in range(NDC):
            nc.sync.dma_start_transpose(
                out=actT[k][:],
                in_=act_d[:, :, k * 128:(k + 1) * 128].rearrange(
                    "b a d -> (b a) d"))

        x1 = small.tile([64, NB * A_], BF16)
        for half in range(2):
            hl = slice(half * 512, (half + 1) * 512)
            px = pY.tile([64, 512], F32, tag="pY", name="pY")
            for k in range(NDC):
                nc.tensor.matmul(out=px[:], lhsT=w1act[k][:], rhs=actT[k][:, hl],
                                 start=(k == 0), stop=(k == NDC - 1))
            for bb in range(4):
                b_idx = half * 4 + bb
                nc.scalar.activation(
                    out=x1[:, b_idx * A_:(b_idx + 1) * A_],
                    in_=px[:, bb * A_:(bb + 1) * A_],
                    func=TANH, bias=latWb[:, b_idx:b_idx + 1], scale=1.0)

        x2 = small.tile([32, NB * A_], BF16)
        for half in range(2):
            hl = slice(half * 512, (half + 1) * 512)
            px = pY.tile([64, 512], F32, tag="pY", name="pY")
            nc.tensor.matmul(out=px[:32, :], lhsT=w2[:], rhs=x1[:, hl],
                             start=True, stop=True)
            nc.scalar.activation(out=x2[:, hl], in_=px[:32, :], func=TANH,
                                 bias=b2[:], scale=1.0)

        x3 = small.tile([1, NB * A_], F32)
        for half in range(2):
            hl = slice(half * 512, (half + 1) * 512)
            px = pY.tile([64, 512], F32, tag="pY", name="pY")
            nc.tensor.matmul(out=px[:1, :], lhsT=w3[:], rhs=x2[:, hl],
                             start=True, stop=True)
            nc.scalar.activation(out=x3[:, hl], in_=px[:1, :], func=TANH,
                                 bias=b3[:], scale=1.0)

        nc.sync.dma_start(out=out_d, in_=x3[:])

    nc.compile()
    return nc


_NC_CACHE = {}


def _get_nc():
    if "nc" not in _NC_CACHE:
        _NC_CACHE["nc"] = _build_nc()
    return _NC_CACHE["nc"]


def _host_tables(nu_log, theta_log, gamma_log, B_re, B_im, C_re, C_im, D,
                 W1, b1, W2, b2, W3, b3):
    f64 = np.float64
    bf = ml_dtypes.bfloat16
    rho_h = np.exp(-np.exp(nu_log.astype(f64)))          # [H]
    theta_h = np.exp(theta_log.astype(f64))              # [H]
    gamma_h = np.exp(gamma_log.astype(f64))              # [H]
    s = np.arange(S_, dtype=f64)
    phase = (theta_h[:, None] * s[None, :]) % (2 * np.pi)   # [H, S]
    cos_t = np.cos(phase)
    sin_t = np.sin(phase)

    def dup(x):  # [H,S] -> [128,S]
        return np.concatenate([x, x], axis=0)

    plane_idx = np.concatenate([np.arange(0, S_, 4), np.arange(2, S_, 4),
                                np.arange(1, S_, 4), np.arange(3, S_, 4)])

    cosS = dup(cos_t).astype(bf)
    sinpm2 = np.concatenate([-sin_t, sin_t], axis=0).astype(bf)
    cos2P = dup(cos_t)[:, plane_idx].astype(bf)
    sin2P = dup(sin_t)[:, plane_idx].astype(bf)

    rho128 = np.concatenate([rho_h, rho_h]).astype(f64)     # [128]
    rho1 = rho128.astype(np.float32).reshape(128, 1)
    rho2 = (rho128 ** 2).astype(np.float32).reshape(128, 1)
    rho4 = np.broadcast_to((rho128 ** 4).astype(np.float32)[:, None],
                           (128, SQ)).copy()

    Bg_re = (B_re.astype(f64) * gamma_h[:, None])        # [H, D_IN]
    Bg_im = (B_im.astype(f64) * gamma_h[:, None])
    statA = np.concatenate([Bg_re.T, Bg_im.T], axis=1)   # [D_IN, 128]
    statA = statA.reshape(NDC, 128, 128).astype(bf)
    permP = np.zeros((128, 128), dtype=bf)
    for m in range(128):
        permP[m ^ 64, m] = 1
    ident = np.eye(128, dtype=bf)
    statD = D.T.reshape(NDC, 128, D_OUT).astype(bf)

    cm1 = np.concatenate([C_re.T, -C_im.T], axis=0).astype(bf)
    cm2 = np.concatenate([-C_im.T, -C_re.T], axis=0).astype(bf)

    w1lat = W1[:, :H_].T.astype(bf)                      # [64 o, 64 m]
    w1act = W1[:, H_:].T.reshape(NDC, 128, D_MLP).astype(bf)
    w2 = W2.T.astype(bf)                                 # [64, 32]
    w3 = W3.T.astype(bf)                                 # [32, 1]

    return dict(
        cosS=cosS, sinpm2=sinpm2, cos2P=cos2P, sin2P=sin2P,
        rho4=rho4, rho1=rho1, rho2=rho2,
        statA=statA, permP=permP, ident=ident, statD=statD, cm1=cm1, cm2=cm2,
        w1lat=w1lat, w1act=w1act, w2=w2, w3=w3,
        b1=b1.reshape(64, 1).astype(np.float32),
        b2=b2.reshape(32, 1).astype(np.float32),
        b3=b3.reshape(1, 1).astype(np.float32),
    )


def kernel(observations, actions, nu_log, theta_log, gamma_log,
           B_re, B_im, C_re, C_im, D, W1, b1, W2, b2, W3, b3,
           _trace=False, _tmpdir=None):
    observations = np.asarray(observations, dtype=np.float32).astype(
        ml_dtypes.bfloat16)
    actions = np.asarray(actions, dtype=np.float32).astype(ml_dtypes.bfloat16)
    tables = _host_tables(np.asarray(nu_log), np.asarray(theta_log),
                          np.asarray(gamma_log), np.asarray(B_re),
                          np.asarray(B_im), np.asarray(C_re),
                          np.asarray(C_im), np.asarray(D),
                          np.asarray(W1), np.asarray(b1), np.asarray(W2),
                          np.asarray(b2), np.asarray(W3), np.asarray(b3))
    in_maps = []
    for c in range(NCORES):
        m = dict(tables)
        m["obs"] = np.ascontiguousarray(observations[c * NB:(c + 1) * NB])
        m["act"] = np.ascontiguousarray(actions[c * NB:(c + 1) * NB])
        in_maps.append(m)

    nc = _get_nc()
    res = run_bass_kernel_spmd(nc, in_maps, core_ids=list(range(NCORES)),
                               trace=_trace, tmpdir=_tmpdir)
    outs = []
    for c in range(NCORES):
        outs.append(np.asarray(res.results[c]["out"]).reshape(NB, A_, 1))
    full = np.concatenate(outs, axis=0).astype(np.float32)
    if _trace:
        return full, res
    return full


# revision 8
# speedup vs baseline: 1.4341x; 1.4341x over previous
"""Trainium2 Bass kernel for nn_ActionScoringModel (LRU + max-pool + tanh MLP).

Strategy: data-parallel over batch (64 = 8 cores x 8 batches). No collectives.
Per core (V2.1 pipeline):
  - obs/act cast to bf16 AND transposed on host -> obsT [NB, 3, 128, S],
    actT [3, 128, NB*A]; device does plain contiguous DMA loads only.
  - u = statA @ obsT, k-major stationary reuse (PSUM, 4 banks)
  - rotate-in: uAc = bf16 copy of u (Act); t1 = uAc (.) cos (DVE),
    t2 = uAc (.) sin' (gpsimd); v = I@t1 + P@t2 on PE (partition swap folded
    into permutation stationary P), Act copies v -> SBUF (padded by 1 col).
  - scan decimation x2: wE_m = rho v_{2m-1} + v_{2m} (stt on DVE);
    hardware scan of length 1024 with rho^2 (DVE); odd positions fixed up
    with one stt: gO = rho gE + v_odd. g layout = [even plane | odd plane]
    which is fine because latent = max over s (permutation invariant).
  - rotate-out: p1 = g (.) cos2P, p2 = g (.) sin2P (plane-ordered tables)
  - y = CM1@p1 + CM2@p2 + D@obsT(strided) per 512-block; two blocks share
    one PSUM bank (partitions 0:64 / 64:128) so each max-reduce covers two
    blocks; final cross-partition pair-max via P matmul + tensor MAX.
  - tanh MLP head on [latent, act].
"""

import sys
import numpy as np
from contextlib import ExitStack

for _p in ("/opt/trn_rl_repo",):
    if _p not in sys.path:
        sys.path.insert(0, _p)

import ml_dtypes
import concourse.bass as bass
import concourse.tile as tile
from concourse import bacc, mybir
from concourse.bass_utils import run_bass_kernel_spmd

BF16 = mybir.dt.bfloat16
F32 = mybir.dt.float32

B_, S_, A_, D_IN, H_, D_OUT, D_MLP = 64, 2048, 128, 384, 64, 64, 64
NCORES = 8
NB = B_ // NCORES          # 8 batches per core
NSB = S_ // 512            # 4 s-blocks of 512
NDC = D_IN // 128          # 3 d-chunks
SH = S_ // 2               # 1024, scan length / plane width


def _build_nc():
    nc = bacc.Bacc("TRN2", target_bir_lowering=False, debug=False,
                   num_devices=1)

    # ---- DRAM I/O ----
    obsT_d = nc.dram_tensor("obsT", [NB, NDC, 128, S_], BF16,
                            kind="ExternalInput").ap()
    actT_d = nc.dram_tensor("actT", [NDC, 128, NB * A_], BF16,
                            kind="ExternalInput").ap()
    cosS_d = nc.dram_tensor("cosS", [128, S_], BF16, kind="ExternalInput").ap()
    sinpm2_d = nc.dram_tensor("sinpm2", [128, S_], BF16, kind="ExternalInput").ap()
    cos2P_d = nc.dram_tensor("cos2P", [128, S_], BF16, kind="ExternalInput").ap()
    sin2P_d = nc.dram_tensor("sin2P", [128, S_], BF16, kind="ExternalInput").ap()
    rho2f_d = nc.dram_tensor("rho2f", [128, SH], F32, kind="ExternalInput").ap()
    rho1_d = nc.dram_tensor("rho1", [128, 1], F32, kind="ExternalInput").ap()
    statA_d = nc.dram_tensor("statA", [NDC, 128, 128], BF16, kind="ExternalInput").ap()
    permP_d = nc.dram_tensor("permP", [128, 128], BF16, kind="ExternalInput").ap()
    ident_d = nc.dram_tensor("ident", [128, 128], BF16, kind="ExternalInput").ap()
    statD_d = nc.dram_tensor("statD", [NDC, 128, D_OUT], BF16, kind="ExternalInput").ap()
    cm1_d = nc.dram_tensor("cm1", [128, D_OUT], BF16, kind="ExternalInput").ap()
    cm2_d = nc.dram_tensor("cm2", [128, D_OUT], BF16, kind="ExternalInput").ap()
    w1lat_d = nc.dram_tensor("w1lat", [64, 64], BF16, kind="ExternalInput").ap()
    w1act_d = nc.dram_tensor("w1act", [NDC, 128, D_MLP], BF16, kind="ExternalInput").ap()
    w2_d = nc.dram_tensor("w2", [64, 32], BF16, kind="ExternalInput").ap()
    w3_d = nc.dram_tensor("w3", [32, 1], BF16, kind="ExternalInput").ap()
    b1_d = nc.dram_tensor("b1", [64, 1], F32, kind="ExternalInput").ap()
    b2_d = nc.dram_tensor("b2", [32, 1], F32, kind="ExternalInput").ap()
    b3_d = nc.dram_tensor("b3", [1, 1], F32, kind="ExternalInput").ap()
    out_d = nc.dram_tensor("out", [1, NB * A_], F32, kind="ExternalOutput").ap()

    MULT = mybir.AluOpType.mult
    ADD = mybir.AluOpType.add
    MAX = mybir.AluOpType.max
    TANH = mybir.ActivationFunctionType.Tanh
    X = mybir.AxisListType.X

    with tile.TileContext(nc) as tc, ExitStack() as ctx:
        const = ctx.enter_context(tc.tile_pool(name="const", bufs=1))
        obsT_pool = ctx.enter_context(tc.tile_pool(name="obsT", bufs=2))
        work = ctx.enter_context(tc.tile_pool(name="work", bufs=2))
        tpool = ctx.enter_context(tc.tile_pool(name="tpool", bufs=3))
        pUA = ctx.enter_context(tc.tile_pool(name="pUA", bufs=1, space="PSUM"))
        pV = ctx.enter_context(tc.tile_pool(name="pV", bufs=2, space="PSUM"))
        pY = ctx.enter_context(tc.tile_pool(name="pY", bufs=1, space="PSUM"))
        small = ctx.enter_context(tc.tile_pool(name="small", bufs=1))

        def load_const(ap_d, shape, dtype, suffix=""):
            nm = f"c_{ap_d.tensor.name}{suffix}"
            t = const.tile(shape, dtype, tag=nm, name=nm)
            nc.scalar.dma_start(out=t[:], in_=ap_d)
            return t

        cosS = load_const(cosS_d, [128, S_], BF16)
        sinpm2 = load_const(sinpm2_d, [128, S_], BF16)
        cos2P = load_const(cos2P_d, [128, S_], BF16)
        sin2P = load_const(sin2P_d, [128, S_], BF16)
        rho2f = load_const(rho2f_d, [128, SH], F32)
        rho1 = load_const(rho1_d, [128, 1], F32)
        statA = [load_const(statA_d[k], [128, 128], BF16, f"{k}") for k in range(NDC)]
        permP = load_const(permP_d, [128, 128], BF16)
        ident = load_const(ident_d, [128, 128], BF16)
        statD = [load_const(statD_d[k], [128, D_OUT], BF16, f"{k}") for k in range(NDC)]
        cm1 = load_const(cm1_d, [128, D_OUT], BF16)
        cm2 = load_const(cm2_d, [128, D_OUT], BF16)
        w1lat = load_const(w1lat_d, [64, 64], BF16)
        w1act = [load_const(w1act_d[k], [128, D_MLP], BF16, f"{k}") for k in range(NDC)]
        w2 = load_const(w2_d, [64, 32], BF16)
        w3 = load_const(w3_d, [32, 1], BF16)
        b1 = load_const(b1_d, [64, 1], F32)
        b2 = load_const(b2_d, [32, 1], F32)
        b3 = load_const(b3_d, [1, 1], F32)

        lat128 = small.tile([128, NB], F32)     # per-pair latent maxima

        # ---------------- main loop over local batches ----------------
        for b in range(NB):
            obsT = [obsT_pool.tile([128, S_], BF16, tag=f"obsT{k}",
                                   name=f"obsT{k}")
                    for k in range(NDC)]
            for k in range(NDC):
                nc.sync.dma_start(out=obsT[k][:], in_=obsT_d[b, k])

            # u = statA @ obsT, k-major (3 weight loads per batch)
            uA = [None] * NSB
            for k in range(NDC):
                for i in range(NSB):
                    if k == 0:
                        uA[i] = pUA.tile([128, 512], F32, tag=f"uA{i}",
                                         name=f"uA{i}")
                    nc.tensor.matmul(
                        out=uA[i][:], lhsT=statA[k][:],
                        rhs=obsT[k][:, i * 512:(i + 1) * 512],
                        start=(k == 0), stop=(k == NDC - 1))

            # rotate-in; v = I@t1 + P@t2 on PE -> PSUM -> SBUF (padded)
            v = work.tile([128, S_ + 1], BF16, tag="v", name="v")
            nc.gpsimd.memset(v[:, 0:1], 0.0)
            for i in range(NSB):
                sl = slice(i * 512, (i + 1) * 512)
                uAc = tpool.tile([128, 512], BF16, tag="uAc", name="uAc")
                nc.scalar.copy(out=uAc[:], in_=uA[i][:])
                t1 = tpool.tile([128, 512], BF16, tag="t1", name="t1")
                t2 = tpool.tile([128, 512], BF16, tag="t2", name="t2")
                nc.vector.tensor_tensor(out=t1[:], in0=uAc[:], in1=cosS[:, sl],
                                        op=MULT)
                nc.gpsimd.tensor_tensor(out=t2[:], in0=uAc[:],
                                        in1=sinpm2[:, sl], op=MULT)
                vps = pV.tile([128, 512], F32, tag="vps", name="vps")
                if i % 2 == 0:
                    nc.tensor.matmul(out=vps[:], lhsT=ident[:], rhs=t1[:],
                                     start=True, stop=False)
                    nc.tensor.matmul(out=vps[:], lhsT=permP[:], rhs=t2[:],
                                     start=False, stop=True)
                else:
                    nc.tensor.matmul(out=vps[:], lhsT=permP[:], rhs=t2[:],
                                     start=True, stop=False)
                    nc.tensor.matmul(out=vps[:], lhsT=ident[:], rhs=t1[:],
                                     start=False, stop=True)
                nc.scalar.copy(out=v[:, 1 + i * 512:1 + (i + 1) * 512],
                               in_=vps[:])

            # decimation x2: wE_m = rho*v_{2m-1} + v_{2m}
            wE = work.tile([128, SH], BF16, tag="wE", name="wE")
            v_lo = v[:, 0:S_].rearrange("p (n f) -> p f n", f=2)[:, 0]
            v_hi = v[:, 1:S_ + 1].rearrange("p (n f) -> p f n", f=2)[:, 0]
            nc.vector.scalar_tensor_tensor(
                out=wE[:], in0=v_lo, scalar=rho1[:], in1=v_hi,
                op0=MULT, op1=ADD)

            # length-1024 scan with rho^2; gE = even plane of g
            g = work.tile([128, S_], BF16, tag="g", name="g")
            nc.vector.tensor_tensor_scan(out=g[:, 0:SH], data0=rho2f[:],
                                         data1=wE[:], initial=0.0,
                                         op0=MULT, op1=ADD)
            # odd fixup: gO = rho*gE + v_odd
            v_odd = v[:, 1:S_ + 1].rearrange("p (n f) -> p f n", f=2)[:, 1]
            nc.vector.scalar_tensor_tensor(
                out=g[:, SH:S_], in0=g[:, 0:SH], scalar=rho1[:], in1=v_odd,
                op0=MULT, op1=ADD)

            # rotate-out (plane-ordered tables)
            p1 = work.tile([128, S_], BF16, tag="p1", name="p1")
            p2 = work.tile([128, S_], BF16, tag="p2", name="p2")
            nc.vector.tensor_tensor(out=p1[:], in0=g[:], in1=cos2P[:], op=MULT)
            nc.vector.tensor_tensor(out=p2[:], in0=g[:], in1=sin2P[:], op=MULT)

            # y = CM1@p1 + CM2@p2 + D@obsT(plane view), two 512-blocks per
            # PSUM bank (partitions 0:64 and 64:128); max-reduce per bank
            py = [pY.tile([128, 512], F32, tag=f"pY{j}", name=f"pY{j}")
                  for j in range(2)]
            # sub-blocks: (plane pl, blk) -> bank=blk, half=pl
            subs = [(pl, blk) for pl in range(2) for blk in range(2)]

            def sub_out(pl, blk):
                return py[blk][pl * 64:(pl + 1) * 64, :]

            def sub_rhs_p(p, pl, blk):
                sl = slice(pl * SH + blk * 512, pl * SH + (blk + 1) * 512)
                return p[:, sl]

            def sub_rhs_obs(k, pl, blk):
                base = obsT[k][:, blk * 1024:(blk + 1) * 1024]
                return base.rearrange("p (n f) -> p f n", f=2)[:, pl]

            for pl, blk in subs:
                nc.tensor.matmul(out=sub_out(pl, blk), lhsT=cm1[:],
                                 rhs=sub_rhs_p(p1, pl, blk),
                                 start=True, stop=False)
            for pl, blk in subs:
                nc.tensor.matmul(out=sub_out(pl, blk), lhsT=cm2[:],
                                 rhs=sub_rhs_p(p2, pl, blk),
                                 start=False, stop=False)
            for k in range(NDC):
                for pl, blk in subs:
                    nc.tensor.matmul(out=sub_out(pl, blk), lhsT=statD[k][:],
                                     rhs=sub_rhs_obs(k, pl, blk),
                                     start=False, stop=(k == NDC - 1))

            ymax = small.tile([128, 2], F32, tag="ymax", name="ymax")
            for j in range(2):
                nc.vector.tensor_reduce(out=ymax[:, j:j + 1], in_=py[j][:],
                                        axis=X, op=MAX)
            nc.vector.tensor_reduce(out=lat128[:, b:b + 1], in_=ymax[:],
                                    axis=X, op=MAX)

        # ---------------- latent pair-max + MLP head ----------------
        lat128b = small.tile([128, NB], BF16)
        nc.vector.tensor_copy(out=lat128b[:], in_=lat128[:])
        pswap = pV.tile([128, 512], F32, tag="vps", name="pswap")
        nc.tensor.matmul(out=pswap[:, :NB], lhsT=permP[:], rhs=lat128b[:],
                         start=True, stop=True)
        latf = small.tile([64, NB], F32)
        nc.vector.tensor_tensor(out=latf[:], in0=lat128[0:64, :],
                                in1=pswap[0:64, :NB], op=MAX)
        latb = small.tile([64, NB], BF16)
        nc.vector.tensor_copy(out=latb[:], in_=latf[:])

        platW = pV.tile([128, 512], F32, tag="vps", name="platW")
        nc.tensor.matmul(out=platW[:64, :NB], lhsT=w1lat[:], rhs=latb[:],
                         start=True, stop=True)
        latWb = small.tile([64, NB], F32)
        nc.vector.tensor_scalar(out=latWb[:], in0=platW[:64, :NB],
                                scalar1=b1[:], scalar2=None, op0=ADD)

        actT = [small.tile([128, NB * A_], BF16, tag=f"actT{k}",
                           name=f"actT{k}") for k in range(NDC)]
        for k in range(NDC):
            nc.sync.dma_start(out=actT[k][:], in_=actT_d[k])

        x1 = small.tile([64, NB * A_], BF16)
        for half in range(2):
            hl = slice(half * 512, (half + 1) * 512)
            px = pV.tile([128, 512], F32, tag="vps", name="px")
            for k in range(NDC):
                nc.tensor.matmul(out=px[:64, :], lhsT=w1act[k][:],
                                 rhs=actT[k][:, hl],
                                 start=(k == 0), stop=(k == NDC - 1))
            for bb in range(4):
                b_idx = half * 4 + bb
                nc.scalar.activation(
                    out=x1[:, b_idx * A_:(b_idx + 1) * A_],
                    in_=px[:64, bb * A_:(bb + 1) * A_],
                    func=TANH, bias=latWb[:, b_idx:b_idx + 1], scale=1.0)

        x2 = small.tile([32, NB * A_], BF16)
        for half in range(2):
            hl = slice(half * 512, (half + 1) * 512)
            px = pV.tile([128, 512], F32, tag="vps", name="px2")
            nc.tensor.matmul(out=px[:32, :], lhsT=w2[:], rhs=x1[:, hl],
                             start=True, stop=True)
            nc.scalar.activation(out=x2[:, hl], in_=px[:32, :], func=TANH,
                                 bias=b2[:], scale=1.0)

        x3 = small.tile([1, NB * A_], F32)
        for half in range(2):
            hl = slice(half * 512, (half + 1) * 512)
            px = pV.tile([128, 512], F32, tag="vps", name="px3")
            nc.tensor.matmul(out=px[:1, :], lhsT=w3[:], rhs=x2[:, hl],
                             start=True, stop=True)
            nc.scalar.activation(out=x3[:, hl], in_=px[:1, :], func=TANH,
                                 bias=b3[:], scale=1.0)

        nc.sync.dma_start(out=out_d, in_=x3[:])

    nc.compile()
    return nc


_NC_CACHE = {}


def _get_nc():
    if "nc" not in _NC_CACHE:
        _NC_CACHE["nc"] = _build_nc()
    return _NC_CACHE["nc"]


def _host_tables(nu_log, theta_log, gamma_log, B_re, B_im, C_re, C_im, D,
                 W1, b1, W2, b2, W3, b3):
    f64 = np.float64
    bf = ml_dtypes.bfloat16
    rho_h = np.exp(-np.exp(nu_log.astype(f64)))          # [H]
    theta_h = np.exp(theta_log.astype(f64))              # [H]
    gamma_h = np.exp(gamma_log.astype(f64))              # [H]
    s = np.arange(S_, dtype=f64)
    phase = (theta_h[:, None] * s[None, :]) % (2 * np.pi)   # [H, S]
    cos_t = np.cos(phase)
    sin_t = np.sin(phase)

    def dup(x):  # [H,S] -> [128,S]
        return np.concatenate([x, x], axis=0)

    plane_idx = np.concatenate([np.arange(0, S_, 2), np.arange(1, S_, 2)])

    cosS = dup(cos_t).astype(bf)
    sinpm2 = np.concatenate([-sin_t, sin_t], axis=0).astype(bf)
    cos2P = dup(cos_t)[:, plane_idx].astype(bf)
    sin2P = dup(sin_t)[:, plane_idx].astype(bf)

    rho128 = np.concatenate([rho_h, rho_h]).astype(f64)     # [128]
    rho1 = rho128.astype(np.float32).reshape(128, 1)
    rho2f = np.broadcast_to((rho128 ** 2).astype(np.float32)[:, None],
                            (128, SH)).copy()

    Bg_re = (B_re.astype(f64) * gamma_h[:, None])        # [H, D_IN]
    Bg_im = (B_im.astype(f64) * gamma_h[:, None])
    statA = np.concatenate([Bg_re.T, Bg_im.T], axis=1)   # [D_IN, 128]
    statA = statA.reshape(NDC, 128, 128).astype(bf)
    permP = np.zeros((128, 128), dtype=bf)
    for m in range(128):
        permP[m ^ 64, m] = 1
    ident = np.eye(128, dtype=bf)
    statD = D.T.reshape(NDC, 128, D_OUT).astype(bf)

    cm1 = np.concatenate([C_re.T, -C_im.T], axis=0).astype(bf)
    cm2 = np.concatenate([-C_im.T, -C_re.T], axis=0).astype(bf)

    w1lat = W1[:, :H_].T.astype(bf)                      # [64 o, 64 m]
    w1act = W1[:, H_:].T.reshape(NDC, 128, D_MLP).astype(bf)
    w2 = W2.T.astype(bf)                                 # [64, 32]
    w3 = W3.T.astype(bf)                                 # [32, 1]

    return dict(
        cosS=cosS, sinpm2=sinpm2, cos2P=cos2P, sin2P=sin2P,
        rho2f=rho2f, rho1=rho1,
        statA=statA, permP=permP, ident=ident, statD=statD, cm1=cm1, cm2=cm2,
        w1lat=w1lat, w1act=w1act, w2=w2, w3=w3,
        b1=b1.reshape(64, 1).astype(np.float32),
        b2=b2.reshape(32, 1).astype(np.float32),
        b3=b3.reshape(1, 1).astype(np.float32),
    )


def kernel(observations, actions, nu_log, theta_log, gamma_log,
           B_re, B_im, C_re, C_im, D, W1, b1, W2, b2, W3, b3,
           _trace=False, _tmpdir=None):
    obs_bf = np.asarray(observations, dtype=np.float32).astype(
        ml_dtypes.bfloat16)
    act_bf = np.asarray(actions, dtype=np.float32).astype(ml_dtypes.bfloat16)
    # host-side transposes: obsT [B, NDC, 128, S]
    obsT_all = np.ascontiguousarray(obs_bf.transpose(0, 2, 1)).reshape(
        B_, NDC, 128, S_)
    tables = _host_tables(np.asarray(nu_log), np.asarray(theta_log),
                          np.asarray(gamma_log), np.asarray(B_re),
                          np.asarray(B_im), np.asarray(C_re),
                          np.asarray(C_im), np.asarray(D),
                          np.asarray(W1), np.asarray(b1), np.asarray(W2),
                          np.asarray(b2), np.asarray(W3), np.asarray(b3))
    in_maps = []
    for c in range(NCORES):
        m = dict(tables)
        m["obsT"] = np.ascontiguousarray(obsT_all[c * NB:(c + 1) * NB])
        act_c = act_bf[c * NB:(c + 1) * NB].reshape(NB * A_, D_IN)
        m["actT"] = np.ascontiguousarray(act_c.T).reshape(NDC, 128, NB * A_)
        in_maps.append(m)

    nc = _get_nc()
    res = run_bass_kernel_spmd(nc, in_maps, core_ids=list(range(NCORES)),
                               trace=_trace, tmpdir=_tmpdir)
    outs = []
    for c in range(NCORES):
        outs.append(np.asarray(res.results[c]["out"]).reshape(NB, A_, 1))
    full = np.concatenate(outs, axis=0).astype(np.float32)
    if _trace:
        return full, res
    return full


# revision 14
# speedup vs baseline: 1.6997x; 1.1853x over previous
"""Trainium2 Bass kernel for nn_ActionScoringModel (LRU + max-pool + tanh MLP).

Strategy: data-parallel over batch (64 = 8 cores x 8 batches). No collectives.
Per core (V2.1 pipeline):
  - obs/act cast to bf16 AND transposed on host -> obsT [NB, 3, 128, S],
    actT [3, 128, NB*A]; device does plain contiguous DMA loads only.
  - u = statA @ obsT, k-major stationary reuse (PSUM, 4 banks)
  - rotate-in: uAc = bf16 copy of u (Act); t1 = uAc (.) cos (DVE),
    t2 = uAc (.) sin' (gpsimd); v = I@t1 + P@t2 on PE (partition swap folded
    into permutation stationary P), Act copies v -> SBUF (padded by 1 col).
  - scan decimation x2: wE_m = rho v_{2m-1} + v_{2m} (stt on DVE);
    hardware scan of length 1024 with rho^2 (DVE); odd positions fixed up
    with one stt: gO = rho gE + v_odd. g layout = [even plane | odd plane]
    which is fine because latent = max over s (permutation invariant).
  - rotate-out: p1 = g (.) cos2P, p2 = g (.) sin2P (plane-ordered tables)
  - y = CM1@p1 + CM2@p2 + D@obsT(strided) per 512-block; two blocks share
    one PSUM bank (partitions 0:64 / 64:128) so each max-reduce covers two
    blocks; final cross-partition pair-max via P matmul + tensor MAX.
  - tanh MLP head on [latent, act].
"""

import sys
import numpy as np
from contextlib import ExitStack

for _p in ("/opt/trn_rl_repo",):
    if _p not in sys.path:
        sys.path.insert(0, _p)

import ml_dtypes
import concourse.bass as bass
import concourse.tile as tile
from concourse import bacc, mybir
from concourse.bass_utils import run_bass_kernel_spmd

BF16 = mybir.dt.bfloat16
F32 = mybir.dt.float32

B_, S_, A_, D_IN, H_, D_OUT, D_MLP = 64, 2048, 128, 384, 64, 64, 64
NCORES = 8
NB = B_ // NCORES          # 8 batches per core
NSB = S_ // 512            # 4 s-blocks of 512
NDC = D_IN // 128          # 3 d-chunks
SH = S_ // 2               # 1024, scan length / plane width


def _build_nc():
    nc = bacc.Bacc("TRN2", target_bir_lowering=False, debug=False,
                   num_devices=1)

    # ---- DRAM I/O ----
    obsT_d = nc.dram_tensor("obsT", [NB, NDC, 128, S_], BF16,
                            kind="ExternalInput").ap()
    actT_d = nc.dram_tensor("actT", [NDC, 128, NB * A_], BF16,
                            kind="ExternalInput").ap()
    tabs_d = nc.dram_tensor("tabs", [128, 4 * S_], BF16, kind="ExternalInput").ap()
    rhopk_d = nc.dram_tensor("rhopk", [128, SH + 1], F32, kind="ExternalInput").ap()
    statpk_d = nc.dram_tensor("statpk", [128, 1216], BF16, kind="ExternalInput").ap()
    w2_d = nc.dram_tensor("w2", [64, 32], BF16, kind="ExternalInput").ap()
    w3_d = nc.dram_tensor("w3", [32, 1], BF16, kind="ExternalInput").ap()
    b1_d = nc.dram_tensor("b1", [64, 1], F32, kind="ExternalInput").ap()
    b2_d = nc.dram_tensor("b2", [32, 1], F32, kind="ExternalInput").ap()
    b3_d = nc.dram_tensor("b3", [1, 1], F32, kind="ExternalInput").ap()
    out_d = nc.dram_tensor("out", [1, NB * A_], F32, kind="ExternalOutput").ap()

    MULT = mybir.AluOpType.mult
    ADD = mybir.AluOpType.add
    MAX = mybir.AluOpType.max
    TANH = mybir.ActivationFunctionType.Tanh
    X = mybir.AxisListType.X

    with tile.TileContext(nc) as tc, ExitStack() as ctx:
        const = ctx.enter_context(tc.tile_pool(name="const", bufs=1))
        obsT_pool = ctx.enter_context(tc.tile_pool(name="obsT", bufs=3))
        work = ctx.enter_context(tc.tile_pool(name="work", bufs=2))
        tpool = ctx.enter_context(tc.tile_pool(name="tpool", bufs=3))
        pUA = ctx.enter_context(tc.tile_pool(name="pUA", bufs=1, space="PSUM"))
        pV = ctx.enter_context(tc.tile_pool(name="pV", bufs=2, space="PSUM"))
        pY = ctx.enter_context(tc.tile_pool(name="pY", bufs=1, space="PSUM"))
        small = ctx.enter_context(tc.tile_pool(name="small", bufs=1))

        def load_const(ap_d, shape, dtype, suffix=""):
            nm = f"c_{ap_d.tensor.name}{suffix}"
            t = const.tile(shape, dtype, tag=nm, name=nm)
            nc.scalar.dma_start(out=t[:], in_=ap_d)
            return t

        # packed consts: stationaries first (small, unblock compute), then
        # big tables split across both hwdge queues
        statpk = const.tile([128, 1216], BF16, tag="statpk", name="statpk")
        nc.scalar.dma_start(out=statpk[:], in_=statpk_d)
        rhopk = const.tile([128, SH + 1], F32, tag="rhopk", name="rhopk")
        nc.sync.dma_start(out=rhopk[:], in_=rhopk_d)
        tabs = const.tile([128, 4 * S_], BF16, tag="tabs", name="tabs")
        nc.scalar.dma_start(out=tabs[:, 0:2 * S_], in_=tabs_d[:, 0:2 * S_])
        nc.sync.dma_start(out=tabs[:, 2 * S_:], in_=tabs_d[:, 2 * S_:])
        w2 = load_const(w2_d, [64, 32], BF16)
        w3 = load_const(w3_d, [32, 1], BF16)
        b1 = load_const(b1_d, [64, 1], F32)
        b2 = load_const(b2_d, [32, 1], F32)
        b3 = load_const(b3_d, [1, 1], F32)

        cosS = tabs[:, 0:S_]
        sinpm2 = tabs[:, S_:2 * S_]
        cos2P = tabs[:, 2 * S_:3 * S_]
        sin2P = tabs[:, 3 * S_:4 * S_]
        rho2f = rhopk[:, 0:SH]
        rho1 = rhopk[:, SH:SH + 1]
        statA = [statpk[:, k * 128:(k + 1) * 128] for k in range(NDC)]
        permP = statpk[:, 384:512]
        ident = statpk[:, 512:640]
        statD = [statpk[:, 640 + k * 64:640 + (k + 1) * 64] for k in range(NDC)]
        cm1 = statpk[:, 832:896]
        cm2 = statpk[:, 896:960]
        w1lat = statpk[:, 960:1024]
        w1act = [statpk[:, 1024 + k * 64:1024 + (k + 1) * 64] for k in range(NDC)]

        lat128 = small.tile([128, NB], F32)     # per-pair latent maxima

        # action-side MLP input (independent of the LRU path): compute
        # xa = W1act @ actT early so the tail only needs activations
        actT = [small.tile([128, NB * A_], BF16, tag=f"actT{k}",
                           name=f"actT{k}") for k in range(NDC)]
        for k in range(NDC):
            nc.sync.dma_start(out=actT[k][:], in_=actT_d[k])
        xa = small.tile([64, NB * A_], F32, tag="xa", name="xa")
        for half in range(2):
            hl = slice(half * 512, (half + 1) * 512)
            pxa = pV.tile([128, 512], F32, tag="vps", name="pxa")
            for k in range(NDC):
                nc.tensor.matmul(out=pxa[:64, :], lhsT=w1act[k],
                                 rhs=actT[k][:, hl],
                                 start=(k == 0), stop=(k == NDC - 1))
            nc.scalar.copy(out=xa[:, hl], in_=pxa[:64, :])

        # ---------------- main loop over local batches ----------------
        for b in range(NB):
            obsT = [obsT_pool.tile([128, S_], BF16, tag=f"obsT{k}",
                                   name=f"obsT{k}")
                    for k in range(NDC)]
            for k in range(NDC):
                nc.sync.dma_start(out=obsT[k][:], in_=obsT_d[b, k])

            # u = statA @ obsT, k-major (3 weight loads per batch)
            uA = [None] * NSB
            for k in range(NDC):
                for i in range(NSB):
                    if k == 0:
                        uA[i] = pUA.tile([128, 512], F32, tag=f"uA{i}",
                                         name=f"uA{i}")
                    nc.tensor.matmul(
                        out=uA[i][:], lhsT=statA[k],
                        rhs=obsT[k][:, i * 512:(i + 1) * 512],
                        start=(k == 0), stop=(k == NDC - 1))

            # rotate-in; v = I@t1 + P@t2 on PE -> PSUM -> SBUF (padded)
            v = work.tile([128, S_ + 1], BF16, tag="v", name="v")
            nc.gpsimd.memset(v[:, 0:1], 0.0)
            for i in range(NSB):
                sl = slice(i * 512, (i + 1) * 512)
                uAc = tpool.tile([128, 512], BF16, tag="uAc", name="uAc")
                nc.scalar.copy(out=uAc[:], in_=uA[i][:])
                t1 = tpool.tile([128, 512], BF16, tag="t1", name="t1")
                t2 = tpool.tile([128, 512], BF16, tag="t2", name="t2")
                nc.vector.tensor_tensor(out=t1[:], in0=uA[i][:], in1=cosS[:, sl],
                                        op=MULT)
                nc.gpsimd.tensor_tensor(out=t2[:], in0=uAc[:],
                                        in1=sinpm2[:, sl], op=MULT)
                vps = pV.tile([128, 512], F32, tag="vps", name="vps")
                if i % 2 == 0:
                    nc.tensor.matmul(out=vps[:], lhsT=ident, rhs=t1[:],
                                     start=True, stop=False)
                    nc.tensor.matmul(out=vps[:], lhsT=permP, rhs=t2[:],
                                     start=False, stop=True)
                else:
                    nc.tensor.matmul(out=vps[:], lhsT=permP, rhs=t2[:],
                                     start=True, stop=False)
                    nc.tensor.matmul(out=vps[:], lhsT=ident, rhs=t1[:],
                                     start=False, stop=True)
                nc.scalar.copy(out=v[:, 1 + i * 512:1 + (i + 1) * 512],
                               in_=vps[:])

            # decimation x2: wE_m = rho*v_{2m-1} + v_{2m}
            wE = work.tile([128, SH], BF16, tag="wE", name="wE")
            v_lo = v[:, 0:S_].rearrange("p (n f) -> p f n", f=2)[:, 0]
            v_hi = v[:, 1:S_ + 1].rearrange("p (n f) -> p f n", f=2)[:, 0]
            nc.vector.scalar_tensor_tensor(
                out=wE[:], in0=v_lo, scalar=rho1, in1=v_hi,
                op0=MULT, op1=ADD)

            # length-1024 scan with rho^2; gE = even plane of g
            g = work.tile([128, S_], BF16, tag="g", name="g")
            nc.vector.tensor_tensor_scan(out=g[:, 0:SH], data0=rho2f,
                                         data1=wE[:], initial=0.0,
                                         op0=MULT, op1=ADD)
            # odd fixup: gO = rho*gE + v_odd
            v_odd = v[:, 1:S_ + 1].rearrange("p (n f) -> p f n", f=2)[:, 1]
            nc.vector.scalar_tensor_tensor(
                out=g[:, SH:S_], in0=g[:, 0:SH], scalar=rho1, in1=v_odd,
                op0=MULT, op1=ADD)

            # rotate-out (plane-ordered tables)
            p1 = work.tile([128, S_], BF16, tag="p1", name="p1")
            p2 = work.tile([128, S_], BF16, tag="p2", name="p2")
            nc.vector.tensor_tensor(out=p1[:], in0=g[:], in1=cos2P, op=MULT)
            nc.gpsimd.tensor_tensor(out=p2[:, 0:SH], in0=g[:, 0:SH],
                                    in1=sin2P[:, 0:SH], op=MULT)
            nc.vector.tensor_tensor(out=p2[:, SH:S_], in0=g[:, SH:S_],
                                    in1=sin2P[:, SH:S_], op=MULT)

            # y = CM1@p1 + CM2@p2 + D@obsT(plane view), two 512-blocks per
            # PSUM bank (partitions 0:64 and 64:128); max-reduce per bank
            py = [pY.tile([128, 512], F32, tag=f"pY{j}", name=f"pY{j}")
                  for j in range(2)]
            # sub-blocks: (plane pl, blk) -> bank=blk, half=pl
            subs = [(pl, blk) for pl in range(2) for blk in range(2)]

            def sub_out(pl, blk):
                return py[blk][pl * 64:(pl + 1) * 64, :]

            def sub_rhs_p(p, pl, blk):
                sl = slice(pl * SH + blk * 512, pl * SH + (blk + 1) * 512)
                return p[:, sl]

            def sub_rhs_obs(k, pl, blk):
                base = obsT[k][:, blk * 1024:(blk + 1) * 1024]
                return base.rearrange("p (n f) -> p f n", f=2)[:, pl]

            for pl, blk in subs:
                nc.tensor.matmul(out=sub_out(pl, blk), lhsT=cm1,
                                 rhs=sub_rhs_p(p1, pl, blk),
                                 start=True, stop=False)
            for pl, blk in subs:
                nc.tensor.matmul(out=sub_out(pl, blk), lhsT=cm2,
                                 rhs=sub_rhs_p(p2, pl, blk),
                                 start=False, stop=False)
            for k in range(NDC):
                for pl, blk in subs:
                    nc.tensor.matmul(out=sub_out(pl, blk), lhsT=statD[k],
                                     rhs=sub_rhs_obs(k, pl, blk),
                                     start=False, stop=(k == NDC - 1))

            ymax = small.tile([128, 2], F32, tag="ymax", name="ymax")
            for j in range(2):
                nc.vector.tensor_reduce(out=ymax[:, j:j + 1], in_=py[j][:],
                                        axis=X, op=MAX)
            nc.vector.tensor_reduce(out=lat128[:, b:b + 1], in_=ymax[:],
                                    axis=X, op=MAX)

        # ---------------- latent pair-max + MLP head ----------------
        lat128b = small.tile([128, NB], BF16)
        nc.vector.tensor_copy(out=lat128b[:], in_=lat128[:])
        pswap = pV.tile([128, 512], F32, tag="vps", name="pswap")
        nc.tensor.matmul(out=pswap[:, :NB], lhsT=permP, rhs=lat128b[:],
                         start=True, stop=True)
        latf = small.tile([64, NB], F32)
        nc.vector.tensor_tensor(out=latf[:], in0=lat128[0:64, :],
                                in1=pswap[0:64, :NB], op=MAX)
        latb = small.tile([64, NB], BF16)
        nc.vector.tensor_copy(out=latb[:], in_=latf[:])

        platW = pV.tile([128, 512], F32, tag="vps", name="platW")
        nc.tensor.matmul(out=platW[:64, :NB], lhsT=w1lat[0:64, :], rhs=latb[:],
                         start=True, stop=True)
        latWb = small.tile([64, NB], F32)
        nc.vector.tensor_scalar(out=latWb[:], in0=platW[:64, :NB],
                                scalar1=b1[:], scalar2=None, op0=ADD)

        x1 = small.tile([64, NB * A_], BF16)
        for b_idx in range(NB):
            nc.scalar.activation(
                out=x1[:, b_idx * A_:(b_idx + 1) * A_],
                in_=xa[:, b_idx * A_:(b_idx + 1) * A_],
                func=TANH, bias=latWb[:, b_idx:b_idx + 1], scale=1.0)

        x2 = small.tile([32, NB * A_], BF16)
        for half in range(2):
            hl = slice(half * 512, (half + 1) * 512)
            px = pV.tile([128, 512], F32, tag="vps", name="px2")
            nc.tensor.matmul(out=px[:32, :], lhsT=w2[:], rhs=x1[:, hl],
                             start=True, stop=True)
            nc.scalar.activation(out=x2[:, hl], in_=px[:32, :], func=TANH,
                                 bias=b2[:], scale=1.0)

        x3 = small.tile([1, NB * A_], F32)
        for half in range(2):
            hl = slice(half * 512, (half + 1) * 512)
            px = pV.tile([128, 512], F32, tag="vps", name="px3")
            nc.tensor.matmul(out=px[:1, :], lhsT=w3[:], rhs=x2[:, hl],
                             start=True, stop=True)
            nc.scalar.activation(out=x3[:, hl], in_=px[:1, :], func=TANH,
                                 bias=b3[:], scale=1.0)

        nc.sync.dma_start(out=out_d, in_=x3[:])

    nc.compile()
    return nc


_NC_CACHE = {}


def _get_nc():
    if "nc" not in _NC_CACHE:
        _NC_CACHE["nc"] = _build_nc()
    return _NC_CACHE["nc"]


def _host_tables(nu_log, theta_log, gamma_log, B_re, B_im, C_re, C_im, D,
                 W1, b1, W2, b2, W3, b3):
    f64 = np.float64
    bf = ml_dtypes.bfloat16
    rho_h = np.exp(-np.exp(nu_log.astype(f64)))          # [H]
    theta_h = np.exp(theta_log.astype(f64))              # [H]
    gamma_h = np.exp(gamma_log.astype(f64))              # [H]
    s = np.arange(S_, dtype=f64)
    phase = (theta_h[:, None] * s[None, :]) % (2 * np.pi)   # [H, S]
    cos_t = np.cos(phase)
    sin_t = np.sin(phase)

    def dup(x):  # [H,S] -> [128,S]
        return np.concatenate([x, x], axis=0)

    plane_idx = np.concatenate([np.arange(0, S_, 2), np.arange(1, S_, 2)])

    cosS = dup(cos_t).astype(bf)
    sinpm2 = np.concatenate([-sin_t, sin_t], axis=0).astype(bf)
    cos2P = dup(cos_t)[:, plane_idx].astype(bf)
    sin2P = dup(sin_t)[:, plane_idx].astype(bf)

    rho128 = np.concatenate([rho_h, rho_h]).astype(f64)     # [128]
    rho1 = rho128.astype(np.float32).reshape(128, 1)
    rho2f = np.broadcast_to((rho128 ** 2).astype(np.float32)[:, None],
                            (128, SH)).copy()

    Bg_re = (B_re.astype(f64) * gamma_h[:, None])        # [H, D_IN]
    Bg_im = (B_im.astype(f64) * gamma_h[:, None])
    statA = np.concatenate([Bg_re.T, Bg_im.T], axis=1)   # [D_IN, 128]
    statA = statA.reshape(NDC, 128, 128).astype(bf)
    permP = np.zeros((128, 128), dtype=bf)
    for m in range(128):
        permP[m ^ 64, m] = 1
    ident = np.eye(128, dtype=bf)
    statD = D.T.reshape(NDC, 128, D_OUT).astype(bf)

    cm1 = np.concatenate([C_re.T, -C_im.T], axis=0).astype(bf)
    cm2 = np.concatenate([-C_im.T, -C_re.T], axis=0).astype(bf)

    w1lat = np.zeros((128, 64), dtype=np.float64)
    w1lat[:H_] = W1[:, :H_].T
    w1lat = w1lat.astype(bf)                             # [128, 64] padded
    w1act = W1[:, H_:].T.reshape(NDC, 128, D_MLP).astype(bf)
    w2 = W2.T.astype(bf)                                 # [64, 32]
    w3 = W3.T.astype(bf)                                 # [32, 1]

    tabs = np.concatenate([cosS, sinpm2, cos2P, sin2P], axis=1)
    rhopk = np.concatenate([rho2f, rho1], axis=1).astype(np.float32)
    # statpk layout: statA(3x128) permP ident statD(3x64) cm1 cm2 w1lat w1act(3x64)
    statpk = np.concatenate(
        [statA.transpose(1, 0, 2).reshape(128, NDC * 128)
         if False else np.concatenate([statA[k] for k in range(NDC)], axis=1),
         permP, ident,
         np.concatenate([statD[k] for k in range(NDC)], axis=1),
         cm1, cm2, w1lat,
         np.concatenate([w1act[k] for k in range(NDC)], axis=1)],
        axis=1).astype(bf)
    assert statpk.shape == (128, 1216), statpk.shape
    return dict(
        tabs=tabs, rhopk=rhopk, statpk=statpk,
        w2=w2, w3=w3,
        b1=b1.reshape(64, 1).astype(np.float32),
        b2=b2.reshape(32, 1).astype(np.float32),
        b3=b3.reshape(1, 1).astype(np.float32),
    )


def kernel(observations, actions, nu_log, theta_log, gamma_log,
           B_re, B_im, C_re, C_im, D, W1, b1, W2, b2, W3, b3,
           _trace=False, _tmpdir=None):
    obs_bf = np.asarray(observations, dtype=np.float32).astype(
        ml_dtypes.bfloat16)
    act_bf = np.asarray(actions, dtype=np.float32).astype(ml_dtypes.bfloat16)
    # host-side transposes: obsT [B, NDC, 128, S]
    obsT_all = np.ascontiguousarray(obs_bf.transpose(0, 2, 1)).reshape(
        B_, NDC, 128, S_)
    tables = _host_tables(np.asarray(nu_log), np.asarray(theta_log),
                          np.asarray(gamma_log), np.asarray(B_re),
                          np.asarray(B_im), np.asarray(C_re),
                          np.asarray(C_im), np.asarray(D),
                          np.asarray(W1), np.asarray(b1), np.asarray(W2),
                          np.asarray(b2), np.asarray(W3), np.asarray(b3))
    in_maps = []
    for c in range(NCORES):
        m = dict(tables)
        m["obsT"] = np.ascontiguousarray(obsT_all[c * NB:(c + 1) * NB])
        act_c = act_bf[c * NB:(c + 1) * NB].reshape(NB * A_, D_IN)
        m["actT"] = np.ascontiguousarray(act_c.T).reshape(NDC, 128, NB * A_)
        in_maps.append(m)

    nc = _get_nc()
    res = run_bass_kernel_spmd(nc, in_maps, core_ids=list(range(NCORES)),
                               trace=_trace, tmpdir=_tmpdir)
    outs = []
    for c in range(NCORES):
        outs.append(np.asarray(res.results[c]["out"]).reshape(NB, A_, 1))
    full = np.concatenate(outs, axis=0).astype(np.float32)
    if _trace:
        return full, res
    return full


# revision 17
# speedup vs baseline: 2.0824x; 1.2251x over previous
"""Trainium2 Bass kernel for nn_ActionScoringModel (LRU + max-pool + tanh MLP).

Strategy: data-parallel over batch (64 = 8 cores x 8 batches). No collectives.
Per core (V2.1 pipeline):
  - obs/act cast to bf16 AND transposed on host -> obsT [NB, 3, 128, S],
    actT [3, 128, NB*A]; device does plain contiguous DMA loads only.
  - u = statA @ obsT, k-major stationary reuse (PSUM, 4 banks)
  - rotate-in: uAc = bf16 copy of u (Act); t1 = uAc (.) cos (DVE),
    t2 = uAc (.) sin' (gpsimd); v = I@t1 + P@t2 on PE (partition swap folded
    into permutation stationary P), Act copies v -> SBUF (padded by 1 col).
  - scan decimation x2: wE_m = rho v_{2m-1} + v_{2m} (stt on DVE);
    hardware scan of length 1024 with rho^2 (DVE); odd positions fixed up
    with one stt: gO = rho gE + v_odd. g layout = [even plane | odd plane]
    which is fine because latent = max over s (permutation invariant).
  - rotate-out: p1 = g (.) cos2P, p2 = g (.) sin2P (plane-ordered tables)
  - y = CM1@p1 + CM2@p2 + D@obsT(strided) per 512-block; two blocks share
    one PSUM bank (partitions 0:64 / 64:128) so each max-reduce covers two
    blocks; final cross-partition pair-max via P matmul + tensor MAX.
  - tanh MLP head on [latent, act].
"""

import sys
import numpy as np
from contextlib import ExitStack

for _p in ("/opt/trn_rl_repo",):
    if _p not in sys.path:
        sys.path.insert(0, _p)

import ml_dtypes
import concourse.bass as bass
import concourse.tile as tile
from concourse import bacc, mybir
from concourse.bass_utils import run_bass_kernel_spmd

BF16 = mybir.dt.bfloat16
F32 = mybir.dt.float32

B_, S_, A_, D_IN, H_, D_OUT, D_MLP = 64, 2048, 128, 384, 64, 64, 64
NCORES = 8
NB = B_ // NCORES          # 8 batches per core
NSB = S_ // 512            # 4 s-blocks of 512
NDC = D_IN // 128          # 3 d-chunks
SH = S_ // 2               # 1024, scan length / plane width


def _build_nc():
    nc = bacc.Bacc("TRN2", target_bir_lowering=False, debug=False,
                   num_devices=1)

    # ---- DRAM I/O ----
    obsT_d = nc.dram_tensor("obsT", [NB, NDC, 128, S_], BF16,
                            kind="ExternalInput").ap()
    actT_d = nc.dram_tensor("actT", [NDC, 128, NB * A_], BF16,
                            kind="ExternalInput").ap()
    tabs_d = nc.dram_tensor("tabs", [128, 3 * S_], BF16, kind="ExternalInput").ap()
    rhopk_d = nc.dram_tensor("rhopk", [128, SH + 1], F32, kind="ExternalInput").ap()
    statpk_d = nc.dram_tensor("statpk", [128, 1600], BF16, kind="ExternalInput").ap()
    w2_d = nc.dram_tensor("w2", [64, 32], BF16, kind="ExternalInput").ap()
    w3_d = nc.dram_tensor("w3", [32, 1], BF16, kind="ExternalInput").ap()
    b1_d = nc.dram_tensor("b1", [64, 1], F32, kind="ExternalInput").ap()
    b2_d = nc.dram_tensor("b2", [32, 1], F32, kind="ExternalInput").ap()
    b3_d = nc.dram_tensor("b3", [1, 1], F32, kind="ExternalInput").ap()
    out_d = nc.dram_tensor("out", [1, NB * A_], F32, kind="ExternalOutput").ap()

    MULT = mybir.AluOpType.mult
    ADD = mybir.AluOpType.add
    MAX = mybir.AluOpType.max
    TANH = mybir.ActivationFunctionType.Tanh
    X = mybir.AxisListType.X

    with tile.TileContext(nc) as tc, ExitStack() as ctx:
        const = ctx.enter_context(tc.tile_pool(name="const", bufs=1))
        obsT_pool = ctx.enter_context(tc.tile_pool(name="obsT", bufs=3))
        work = ctx.enter_context(tc.tile_pool(name="work", bufs=2))
        tpool = ctx.enter_context(tc.tile_pool(name="tpool", bufs=3))
        pUA = ctx.enter_context(tc.tile_pool(name="pUA", bufs=1, space="PSUM"))
        pWE = ctx.enter_context(tc.tile_pool(name="pWE", bufs=1, space="PSUM"))
        pY = ctx.enter_context(tc.tile_pool(name="pY", bufs=1, space="PSUM"))
        small = ctx.enter_context(tc.tile_pool(name="small", bufs=1))

        def load_const(ap_d, shape, dtype, suffix=""):
            nm = f"c_{ap_d.tensor.name}{suffix}"
            t = const.tile(shape, dtype, tag=nm, name=nm)
            nc.scalar.dma_start(out=t[:], in_=ap_d)
            return t

        # packed consts: stationaries first (small, unblock compute), then
        # big tables split across both hwdge queues
        statpk = const.tile([128, 1600], BF16, tag="statpk", name="statpk")
        nc.scalar.dma_start(out=statpk[:], in_=statpk_d)
        rhopk = const.tile([128, SH + 1], F32, tag="rhopk", name="rhopk")
        nc.sync.dma_start(out=rhopk[:], in_=rhopk_d)
        tabs = const.tile([128, 3 * S_], BF16, tag="tabs", name="tabs")
        nc.scalar.dma_start(out=tabs[:, 0:2 * S_], in_=tabs_d[:, 0:2 * S_])
        nc.sync.dma_start(out=tabs[:, 2 * S_:], in_=tabs_d[:, 2 * S_:])
        w2 = load_const(w2_d, [64, 32], BF16)
        w3 = load_const(w3_d, [32, 1], BF16)
        b1 = load_const(b1_d, [64, 1], F32)
        b2 = load_const(b2_d, [32, 1], F32)
        b3 = load_const(b3_d, [1, 1], F32)

        cosS = tabs[:, 0:S_]
        sinpm2 = tabs[:, S_:2 * S_]
        cosE = tabs[:, 2 * S_:2 * S_ + SH]
        sinE = tabs[:, 2 * S_ + SH:3 * S_]
        rho2f = rhopk[:, 0:SH]
        rho1 = rhopk[:, SH:SH + 1]
        statA = [statpk[:, k * 128:(k + 1) * 128] for k in range(NDC)]
        permP = statpk[:, 384:512]
        ident = statpk[:, 512:640]
        statD = [statpk[:, 640 + k * 64:640 + (k + 1) * 64] for k in range(NDC)]
        cm1 = statpk[:, 832:896]
        cm2 = statpk[:, 896:960]
        w1lat = statpk[:, 960:1024]
        w1act = [statpk[:, 1024 + k * 64:1024 + (k + 1) * 64] for k in range(NDC)]
        rhoI = statpk[:, 1216:1344]
        rhoP = statpk[:, 1344:1472]
        cm1l = statpk[:, 1472:1536]
        cm2l = statpk[:, 1536:1600]

        lat128 = small.tile([128, NB], F32)     # per-pair latent maxima

        # action-side MLP input (independent of the LRU path): compute
        # xa = W1act @ actT early so the tail only needs activations
        actT = [small.tile([128, NB * A_], BF16, tag=f"actT{k}",
                           name=f"actT{k}") for k in range(NDC)]
        for k in range(NDC):
            nc.sync.dma_start(out=actT[k][:], in_=actT_d[k])
        xa = small.tile([64, NB * A_], F32, tag="xa", name="xa")
        for half in range(2):
            hl = slice(half * 512, (half + 1) * 512)
            pxa = pWE.tile([128, 512], F32, tag="wE0", name="pxa")
            for k in range(NDC):
                nc.tensor.matmul(out=pxa[:64, :], lhsT=w1act[k],
                                 rhs=actT[k][:, hl],
                                 start=(k == 0), stop=(k == NDC - 1))
            nc.scalar.copy(out=xa[:, hl], in_=pxa[:64, :])

        # ---------------- main loop over local batches ----------------
        for b in range(NB):
            obsT = [obsT_pool.tile([128, S_], BF16, tag=f"obsT{k}",
                                   name=f"obsT{k}")
                    for k in range(NDC)]
            for k in range(NDC):
                nc.sync.dma_start(out=obsT[k][:], in_=obsT_d[b, k])

            # u = statA @ obsT, k-major (3 weight loads per batch)
            uA = [None] * NSB
            for k in range(NDC):
                for i in range(NSB):
                    if k == 0:
                        uA[i] = pUA.tile([128, 512], F32, tag=f"uA{i}",
                                         name=f"uA{i}")
                    nc.tensor.matmul(
                        out=uA[i][:], lhsT=statA[k],
                        rhs=obsT[k][:, i * 512:(i + 1) * 512],
                        start=(k == 0), stop=(k == NDC - 1))

            # rotate-in into padded full-batch tiles (col0 = 0)
            t1 = work.tile([128, S_ + 1], BF16, tag="t1", name="t1")
            t2 = work.tile([128, S_ + 1], BF16, tag="t2", name="t2")
            nc.gpsimd.memset(t1[:, 0:1], 0.0)
            nc.gpsimd.memset(t2[:, 0:1], 0.0)
            uAc = work.tile([128, S_], BF16, tag="uAc", name="uAc")
            for i in range(NSB):
                sl = slice(i * 512, (i + 1) * 512)
                slp = slice(1 + i * 512, 1 + (i + 1) * 512)
                nc.scalar.copy(out=uAc[:, sl], in_=uA[i][:])
                nc.vector.tensor_tensor(out=t1[:, slp], in0=uA[i][:],
                                        in1=cosS[:, sl], op=MULT)
                nc.gpsimd.tensor_tensor(out=t2[:, slp], in0=uAc[:, sl],
                                        in1=sinpm2[:, sl], op=MULT)

            # wE = rhoI@t1_odd + rhoP@t2_odd + I@t1_even + P@t2_even on PE
            # (wE_m = rho*v_{2m-1} + v_{2m}, v = I@t1 + P@t2)
            t1_lo = t1[:, 0:S_].rearrange("p (n f) -> p f n", f=2)[:, 0]
            t1_hi = t1[:, 1:S_ + 1].rearrange("p (n f) -> p f n", f=2)[:, 0]
            t2_lo = t2[:, 0:S_].rearrange("p (n f) -> p f n", f=2)[:, 0]
            t2_hi = t2[:, 1:S_ + 1].rearrange("p (n f) -> p f n", f=2)[:, 0]
            wE = [pWE.tile([128, 512], F32, tag=f"wE{j}", name=f"wE{j}")
                  for j in range(2)]
            for j in range(2):
                jl = slice(j * 512, (j + 1) * 512)
                nc.tensor.matmul(out=wE[j][:], lhsT=rhoI, rhs=t1_lo[:, jl],
                                 start=True, stop=False)
                nc.tensor.matmul(out=wE[j][:], lhsT=rhoP, rhs=t2_lo[:, jl],
                                 start=False, stop=False)
                nc.tensor.matmul(out=wE[j][:], lhsT=ident, rhs=t1_hi[:, jl],
                                 start=False, stop=False)
                nc.tensor.matmul(out=wE[j][:], lhsT=permP, rhs=t2_hi[:, jl],
                                 start=False, stop=True)

            # chained length-512 scans with rho^2 over the wE PSUM banks
            g = work.tile([128, SH], BF16, tag="g", name="g")
            nc.vector.tensor_tensor_scan(out=g[:, 0:512], data0=rho2f[:, 0:512],
                                         data1=wE[0][:], initial=0.0,
                                         op0=MULT, op1=ADD)
            nc.vector.tensor_tensor_scan(out=g[:, 512:SH],
                                         data0=rho2f[:, 512:SH],
                                         data1=wE[1][:],
                                         initial=g[:, 511:512],
                                         op0=MULT, op1=ADD)

            # rotate-out (even positions only)
            p1 = work.tile([128, SH], BF16, tag="p1", name="p1")
            p2 = work.tile([128, SH], BF16, tag="p2", name="p2")
            nc.vector.tensor_tensor(out=p1[:], in0=g[:], in1=cosE[:], op=MULT)
            nc.vector.tensor_tensor(out=p2[:], in0=g[:], in1=sinE[:], op=MULT)

            # y even blocks: cm1@p1 + cm2@p2 + statD@obsT_even
            # y odd blocks:  cm1l@p1 + cm2l@p2 + cm1@u_odd + statD@obsT_odd
            # (pl, blk): pl 0=even (s=2m) half [0:64], 1=odd (s=2m+1) [64:128]
            py = [pY.tile([128, 512], F32, tag=f"pY{j}", name=f"pY{j}")
                  for j in range(2)]
            subs = [(pl, blk) for pl in range(2) for blk in range(2)]

            def sub_out(pl, blk):
                return py[blk][pl * 64:(pl + 1) * 64, :]

            uAc_odd = uAc[:].rearrange("p (n f) -> p f n", f=2)[:, 1]
            for pl, blk in subs:
                jl = slice(blk * 512, (blk + 1) * 512)
                nc.tensor.matmul(out=sub_out(pl, blk),
                                 lhsT=(cm1 if pl == 0 else cm1l),
                                 rhs=p1[:, jl], start=True, stop=False)
                nc.tensor.matmul(out=sub_out(pl, blk),
                                 lhsT=(cm2 if pl == 0 else cm2l),
                                 rhs=p2[:, jl], start=False, stop=False)
                if pl == 1:
                    nc.tensor.matmul(out=sub_out(pl, blk), lhsT=cm1,
                                     rhs=uAc_odd[:, jl], start=False,
                                     stop=False)
                for k in range(NDC):
                    base = obsT[k][:, blk * 1024:(blk + 1) * 1024]
                    obsP = base.rearrange("p (n f) -> p f n", f=2)[:, pl]
                    nc.tensor.matmul(out=sub_out(pl, blk), lhsT=statD[k],
                                     rhs=obsP, start=False,
                                     stop=(k == NDC - 1))

            ymax = small.tile([128, 2], F32, tag="ymax", name="ymax")
            for j in range(2):
                nc.vector.tensor_reduce(out=ymax[:, j:j + 1], in_=py[j][:],
                                        axis=X, op=MAX)
            nc.vector.tensor_reduce(out=lat128[:, b:b + 1], in_=ymax[:],
                                    axis=X, op=MAX)

        # ---------------- latent pair-max + MLP head ----------------
        lat128b = small.tile([128, NB], BF16)
        nc.vector.tensor_copy(out=lat128b[:], in_=lat128[:])
        pswap = pWE.tile([128, 512], F32, tag="wE0", name="pswap")
        nc.tensor.matmul(out=pswap[:, :NB], lhsT=permP, rhs=lat128b[:],
                         start=True, stop=True)
        latf = small.tile([64, NB], F32)
        nc.vector.tensor_tensor(out=latf[:], in0=lat128[0:64, :],
                                in1=pswap[0:64, :NB], op=MAX)
        latb = small.tile([64, NB], BF16)
        nc.vector.tensor_copy(out=latb[:], in_=latf[:])

        platW = pWE.tile([128, 512], F32, tag="wE0", name="platW")
        nc.tensor.matmul(out=platW[:64, :NB], lhsT=w1lat[0:64, :], rhs=latb[:],
                         start=True, stop=True)
        latWb = small.tile([64, NB], F32)
        nc.vector.tensor_scalar(out=latWb[:], in0=platW[:64, :NB],
                                scalar1=b1[:], scalar2=None, op0=ADD)

        x1 = small.tile([64, NB * A_], BF16)
        for b_idx in range(NB):
            nc.scalar.activation(
                out=x1[:, b_idx * A_:(b_idx + 1) * A_],
                in_=xa[:, b_idx * A_:(b_idx + 1) * A_],
                func=TANH, bias=latWb[:, b_idx:b_idx + 1], scale=1.0)

        x2 = small.tile([32, NB * A_], BF16)
        for half in range(2):
            hl = slice(half * 512, (half + 1) * 512)
            px = pWE.tile([128, 512], F32, tag="wE0", name="px2")
            nc.tensor.matmul(out=px[:32, :], lhsT=w2[:], rhs=x1[:, hl],
                             start=True, stop=True)
            nc.scalar.activation(out=x2[:, hl], in_=px[:32, :], func=TANH,
                                 bias=b2[:], scale=1.0)

        x3 = small.tile([1, NB * A_], F32)
        for half in range(2):
            hl = slice(half * 512, (half + 1) * 512)
            px = pWE.tile([128, 512], F32, tag="wE0", name="px3")
            nc.tensor.matmul(out=px[:1, :], lhsT=w3[:], rhs=x2[:, hl],
                             start=True, stop=True)
            nc.scalar.activation(out=x3[:, hl], in_=px[:1, :], func=TANH,
                                 bias=b3[:], scale=1.0)

        nc.sync.dma_start(out=out_d, in_=x3[:])

    nc.compile()
    return nc


_NC_CACHE = {}


def _get_nc():
    if "nc" not in _NC_CACHE:
        _NC_CACHE["nc"] = _build_nc()
    return _NC_CACHE["nc"]


def _host_tables(nu_log, theta_log, gamma_log, B_re, B_im, C_re, C_im, D,
                 W1, b1, W2, b2, W3, b3):
    f64 = np.float64
    bf = ml_dtypes.bfloat16
    rho_h = np.exp(-np.exp(nu_log.astype(f64)))          # [H]
    theta_h = np.exp(theta_log.astype(f64))              # [H]
    gamma_h = np.exp(gamma_log.astype(f64))              # [H]
    s = np.arange(S_, dtype=f64)
    phase = (theta_h[:, None] * s[None, :]) % (2 * np.pi)   # [H, S]
    cos_t = np.cos(phase)
    sin_t = np.sin(phase)

    def dup(x):  # [H,S] -> [128,S]
        return np.concatenate([x, x], axis=0)

    cosS = dup(cos_t).astype(bf)
    sinpm2 = np.concatenate([-sin_t, sin_t], axis=0).astype(bf)
    cosE = dup(cos_t)[:, 0::2].astype(bf)
    sinE = dup(sin_t)[:, 0::2].astype(bf)

    rho128 = np.concatenate([rho_h, rho_h]).astype(f64)     # [128]
    rho1 = rho128.astype(np.float32).reshape(128, 1)
    rho2f = np.broadcast_to((rho128 ** 2).astype(np.float32)[:, None],
                            (128, SH)).copy()

    Bg_re = (B_re.astype(f64) * gamma_h[:, None])        # [H, D_IN]
    Bg_im = (B_im.astype(f64) * gamma_h[:, None])
    statA = np.concatenate([Bg_re.T, Bg_im.T], axis=1)   # [D_IN, 128]
    statA = statA.reshape(NDC, 128, 128).astype(bf)
    permP = np.zeros((128, 128), dtype=bf)
    for m in range(128):
        permP[m ^ 64, m] = 1
    ident = np.eye(128, dtype=bf)
    statD = D.T.reshape(NDC, 128, D_OUT).astype(bf)

    cm1 = np.concatenate([C_re.T, -C_im.T], axis=0).astype(bf)
    cm2 = np.concatenate([-C_im.T, -C_re.T], axis=0).astype(bf)
    inC_re = C_re.astype(f64)
    inC_im = C_im.astype(f64)

    w1lat = np.zeros((128, 64), dtype=np.float64)
    w1lat[:H_] = W1[:, :H_].T
    w1lat = w1lat.astype(bf)                             # [128, 64] padded
    w1act = W1[:, H_:].T.reshape(NDC, 128, D_MLP).astype(bf)
    w2 = W2.T.astype(bf)                                 # [64, 32]
    w3 = W3.T.astype(bf)                                 # [32, 1]

    tabs = np.concatenate([cosS, sinpm2, cosE, sinE], axis=1)
    rhopk = np.concatenate([rho2f, rho1], axis=1).astype(np.float32)
    # statpk layout: statA(3x128) permP ident statD(3x64) cm1 cm2 w1lat w1act(3x64)
    # lambda-folded C: C' = C * diag(lambda)
    lam_re = (rho128[:H_] * np.cos(theta_h))
    lam_im = (rho128[:H_] * np.sin(theta_h))
    Cp_re = inC_re * lam_re[None, :] - inC_im * lam_im[None, :]
    Cp_im = inC_re * lam_im[None, :] + inC_im * lam_re[None, :]
    cm1l = np.concatenate([Cp_re.T, -Cp_im.T], axis=0).astype(bf)
    cm2l = np.concatenate([-Cp_im.T, -Cp_re.T], axis=0).astype(bf)
    rhoI = (np.eye(128) * rho128[None, :]).astype(bf)
    rhoP = (permP.astype(np.float64) * rho128[None, :]).astype(bf)
    statpk = np.concatenate(
        [np.concatenate([statA[k] for k in range(NDC)], axis=1),
         permP, ident,
         np.concatenate([statD[k] for k in range(NDC)], axis=1),
         cm1, cm2, w1lat,
         np.concatenate([w1act[k] for k in range(NDC)], axis=1),
         rhoI, rhoP, cm1l, cm2l],
        axis=1).astype(bf)
    assert statpk.shape == (128, 1600), statpk.shape
    return dict(
        tabs=tabs, rhopk=rhopk, statpk=statpk,
        w2=w2, w3=w3,
        b1=b1.reshape(64, 1).astype(np.float32),
        b2=b2.reshape(32, 1).astype(np.float32),
        b3=b3.reshape(1, 1).astype(np.float32),
    )


def kernel(observations, actions, nu_log, theta_log, gamma_log,
           B_re, B_im, C_re, C_im, D, W1, b1, W2, b2, W3, b3,
           _trace=False, _tmpdir=None):
    obs_bf = np.asarray(observations, dtype=np.float32).astype(
        ml_dtypes.bfloat16)
    act_bf = np.asarray(actions, dtype=np.float32).astype(ml_dtypes.bfloat16)
    # host-side transposes: obsT [B, NDC, 128, S]
    obsT_all = np.ascontiguousarray(obs_bf.transpose(0, 2, 1)).reshape(
        B_, NDC, 128, S_)
    tables = _host_tables(np.asarray(nu_log), np.asarray(theta_log),
                          np.asarray(gamma_log), np.asarray(B_re),
                          np.asarray(B_im), np.asarray(C_re),
                          np.asarray(C_im), np.asarray(D),
                          np.asarray(W1), np.asarray(b1), np.asarray(W2),
                          np.asarray(b2), np.asarray(W3), np.asarray(b3))
    in_maps = []
    for c in range(NCORES):
        m = dict(tables)
        m["obsT"] = np.ascontiguousarray(obsT_all[c * NB:(c + 1) * NB])
        act_c = act_bf[c * NB:(c + 1) * NB].reshape(NB * A_, D_IN)
        m["actT"] = np.ascontiguousarray(act_c.T).reshape(NDC, 128, NB * A_)
        in_maps.append(m)

    nc = _get_nc()
    res = run_bass_kernel_spmd(nc, in_maps, core_ids=list(range(NCORES)),
                               trace=_trace, tmpdir=_tmpdir)
    outs = []
    for c in range(NCORES):
        outs.append(np.asarray(res.results[c]["out"]).reshape(NB, A_, 1))
    full = np.concatenate(outs, axis=0).astype(np.float32)
    if _trace:
        return full, res
    return full
